# revision 1
# baseline (speedup 1.0000x reference)
import sys

sys.path.insert(0, "/opt/trn_rl_repo")

import numpy as np
import ml_dtypes

# ---------------- constants (hardcoded problem geometry) ----------------
B, C, H, W = 4, 64, 256, 256
HEADS = 4
N_CORES = 8
R = 128             # sample rows per core (H split in 2)
WB = W + 10         # padded width 266
BLK = 16            # output rows per block
NKVB = R // BLK     # 8 kv blocks
NQB = (R // 2) // BLK  # 4 q blocks (packed halves)
SRC_R = BLK + 10    # 26 src/a0 rows per block
A1_R = BLK + 6      # 22 a1 content rows
A0F = SRC_R * WB    # 6916
A1F = A1_R * WB     # 5852
A2F = BLK * WB      # 4256
NKV = R * W         # 32768
NQ = (R // 2) * W   # 16384
GN_EPS = 1e-5


def d5_off(t):
    return (t // 5) * WB + (t % 5)


def d3_off(t):
    # a1 column basis: data col = j - 3  ->  col offset 3*kw - 5
    return WB + (t // 3) * 3 * WB + ((t % 3) * 3 - 5)


# tap assignment: DVE keeps only 4B-aligned (even-offset) taps for 2x mode;
# PE takes all odd-offset taps plus extra even ones for engine balance.
_odd5 = [t for t in range(25) if (t % 5) in (1, 3)]
_ev5 = [t for t in range(25) if (t % 5) in (0, 2, 4)]
PE5 = _odd5 + [_ev5[0], _ev5[4], _ev5[10], _ev5[14]]         # 14
DVE5 = [t for t in _ev5 if t not in PE5]                     # 11
GP5 = []
PE3 = [0, 2, 3, 5, 6, 8]   # odd-offset taps (kw!=1) + balance
DVE3 = [1, 4, 7]           # kw==1 -> even offset -> 2x eligible
GP3 = []

_CACHE = {}


def _build():
    if "nc" in _CACHE:
        return _CACHE["nc"]
    import concourse.bacc as bacc
    import concourse.tile as tile
    from concourse import mybir

    BF = mybir.dt.bfloat16
    F32 = mybir.dt.float32
    AF = mybir.ActivationFunctionType
    OP = mybir.AluOpType
    AX = mybir.AxisListType

    nc = bacc.Bacc("TRN2", target_bir_lowering=False, debug=False,
                   num_devices=N_CORES)

    def din(name, shape, dt=F32):
        return nc.dram_tensor(name, shape, dt, kind="ExternalInput").ap()

    ysl = din("ysl", [C, (R + 10) * WB], BF)
    xpk = din("xpk", [128, (R // 2 + 10) * WB], BF)
    kvwT = din("kvwT", [C, 128], BF)
    kv1wT = din("kv1wT", [128, 128], BF)
    qwT2 = din("qwT2", [128, 128], BF)
    q1wT2 = din("q1wT2", [128, 128], BF)
    d5kv = din("d5kv", [128, len(PE5) * 128], BF)
    d3kv = din("d3kv", [128, len(PE3) * 128], BF)
    d5q = din("d5q", [128, len(PE5) * 128], BF)
    d3q = din("d3q", [128, len(PE3) * 128], BF)
    w5kv = din("w5kv", [128, 25])
    w3kv = din("w3kv", [128, 9])
    w5q = din("w5q", [128, 25])
    w3q = din("w3q", [128, 9])
    bkv0 = din("bkv0", [128, 1])
    bkvs = din("bkvs", [128, 1])
    bkv1 = din("bkv1", [128, 1])
    bq0 = din("bq0", [128, 1])
    bqs = din("bqs", [128, 1])
    bq1 = din("bq1", [128, 1])
    m0t_kv = din("m0t_kv", [128, 1])
    m0b_kv = din("m0b_kv", [128, 1])
    m0t_q = din("m0t_q", [128, 1])
    m0b_q = din("m0b_q", [128, 1])
    g_kv = din("g_kv", [128, 1])
    be_kv = din("be_kv", [128, 1])
    g_q = din("g_q", [128, 1])
    be_q = din("be_q", [128, 1])
    ind = din("ind", [128, 4])
    bc_kv = din("bc_kv", [4, 128])
    bc_q = din("bc_q", [4, 128])
    cntr = din("cntr", [4, 1])
    tau64 = din("tau64", [64, 1])
    bmask = din("bmask", [64, 64])
    idn = din("idn", [128, 128], BF)
    idnf = din("idnf", [64, 64])
    projT = din("projT", [64, 64])
    out_d = nc.dram_tensor("out", [C, NKV], F32, kind="ExternalOutput").ap()

    def ceil(a, b):
        return (a + b - 1) // b

    with tile.TileContext(nc) as tc:
        with (
            tc.tile_pool(name="big", bufs=4) as pbig,
            tc.tile_pool(name="a1p", bufs=2) as pa1,
            tc.tile_pool(name="pers", bufs=1) as pers,
            tc.tile_pool(name="wts", bufs=1) as pwts,
            tc.tile_pool(name="tiny", bufs=2) as ptiny,
            tc.tile_pool(name="tchk", bufs=6) as ptchk,
            tc.tile_pool(name="osbp", bufs=3) as posb,
            tc.tile_pool(name="ps", bufs=4, space="PSUM") as pps,
            tc.tile_pool(name="psT", bufs=3, space="PSUM") as ppsT,
            tc.tile_pool(name="psG", bufs=1, space="PSUM") as ppsG,
            tc.tile_pool(name="dram", bufs=1, space="DRAM") as pdram,
        ):
            a3kv = pers.tile([128, NKV], BF)
            a3qp = pers.tile([128, NQ], BF)
            accA = pers.tile([128, 96], F32)
            sqA = pers.tile([128, 12], F32)
            av2 = pers.tile([128, 66], F32)

            def wtile(src):
                t = pwts.tile(list(src.shape), src.dtype,
                              tag="w_" + src.tensor.name)
                nc.sync.dma_start(out=t[:], in_=src[:])
                return t

            s_kvwT = wtile(kvwT)
            s_kv1wT = wtile(kv1wT)
            s_qwT2 = wtile(qwT2)
            s_q1wT2 = wtile(q1wT2)
            s_d5kv, s_d3kv = wtile(d5kv), wtile(d3kv)
            s_d5q, s_d3q = wtile(d5q), wtile(d3q)
            s_w5kv, s_w3kv = wtile(w5kv), wtile(w3kv)
            s_w5q, s_w3q = wtile(w5q), wtile(w3q)
            s_bkv0, s_bkvs, s_bkv1 = wtile(bkv0), wtile(bkvs), wtile(bkv1)
            s_bq0, s_bqs, s_bq1 = wtile(bq0), wtile(bqs), wtile(bq1)
            s_m0t_kv, s_m0b_kv = wtile(m0t_kv), wtile(m0b_kv)
            s_m0t_q, s_m0b_q = wtile(m0t_q), wtile(m0b_q)
            s_gkv, s_bekv = wtile(g_kv), wtile(be_kv)
            s_gq, s_beq = wtile(g_q), wtile(be_q)
            s_ind, s_bckv, s_bcq = wtile(ind), wtile(bc_kv), wtile(bc_q)
            s_cntr, s_tau, s_bmask = wtile(cntr), wtile(tau64), wtile(bmask)
            s_idn, s_projT = wtile(idn), wtile(projT)
            s_idnf = wtile(idnf)

            acc_col = [0]

            def do_block(src_dram, src_row0, K, c1wA, c1wB, d5, d3, w5, w3,
                         b0, bs, b1, first, last, mt, mb, a3dst, a3off):
                src = pbig.tile([128, SRC_R, WB], BF, tag="big")
                nc.sync.dma_start(
                    out=src[:K].rearrange("p r c -> p (r c)"),
                    in_=src_dram[:, src_row0 * WB:(src_row0 + SRC_R) * WB])
                srcf = src.rearrange("p r c -> p (r c)")
                # stage A: conv1x1 -> a0
                a0 = pbig.tile([128, A0F + 16], BF, tag="big")
                a0f = a0
                for k in range(ceil(A0F, 512)):
                    n = min(512, A0F - k * 512)
                    ps = pps.tile([128, 512], F32)
                    nc.tensor.matmul(ps[:, :n], c1wA[:K],
                                     srcf[:K, k * 512:k * 512 + n],
                                     start=True, stop=True)
                    nc.scalar.copy(a0f[:, k * 512:k * 512 + n], ps[:, :n])
                # stage B: dw5x5 -> a1
                a1 = pa1.tile([128, A1_R + 2, WB], BF, tag="a1")
                a1f = a1.rearrange("p r c -> p (r c)")
                a1c = a1f[:, WB:WB + A1F]
                for k in range(ceil(A1F, 512)):
                    n = min(512, A1F - k * 512)
                    ps = pps.tile([128, 512], F32)
                    for j, t in enumerate(PE5):
                        nc.tensor.matmul(
                            ps[:, :n], d5[:, j * 128:(j + 1) * 128],
                            a0f[:, k * 512 + d5_off(t):k * 512 + d5_off(t) + n],
                            start=(j == 0), stop=(j == len(PE5) - 1))
                    nc.scalar.activation(a1f[:, WB + k * 512:WB + k * 512 + n],
                                         ps[:, :n], AF.Identity, bias=b0)
                for t in DVE5:
                    nc.vector.scalar_tensor_tensor(
                        a1c, a0f[:, d5_off(t):d5_off(t) + A1F], w5[:, t:t + 1],
                        a1c, OP.mult, OP.add)
                for t in GP5:
                    nc.gpsimd.scalar_tensor_tensor(
                        a1c, a0f[:, d5_off(t):d5_off(t) + A1F], w5[:, t:t + 1],
                        a1c, OP.mult, OP.add)
                if first:
                    nc.vector.tensor_scalar_mul(a1f[:, WB:WB + 3 * WB],
                                                a1f[:, WB:WB + 3 * WB], mt)
                if last:
                    lo = WB + (A1_R - 3) * WB
                    nc.vector.tensor_scalar_mul(a1f[:, lo:lo + 3 * WB],
                                                a1f[:, lo:lo + 3 * WB], mb)
                nc.gpsimd.memset(a1[:, 1:, 0:3], 0.0)
                nc.gpsimd.memset(a1[:, 1:, 259:266], 0.0)
                # stage C: dw3x3 dil3 -> a2
                a2 = pbig.tile([128, SRC_R, WB], BF, tag="big")
                a2f = a2.rearrange("p r c -> p (r c)")
                for k in range(ceil(A2F, 512)):
                    n = min(512, A2F - k * 512)
                    ps = pps.tile([128, 512], F32)
                    for j, t in enumerate(PE3):
                        nc.tensor.matmul(
                            ps[:, :n], d3[:, j * 128:(j + 1) * 128],
                            a1f[:, k * 512 + d3_off(t):k * 512 + d3_off(t) + n],
                            start=(j == 0), stop=(j == len(PE3) - 1))
                    nc.scalar.activation(a2f[:, k * 512:k * 512 + n],
                                         ps[:, :n], AF.Identity, bias=bs)
                for t in DVE3:
                    nc.vector.scalar_tensor_tensor(
                        a2f[:, :A2F], a1f[:, d3_off(t):d3_off(t) + A2F],
                        w3[:, t:t + 1], a2f[:, :A2F], OP.mult, OP.add)
                for t in GP3:
                    nc.gpsimd.scalar_tensor_tensor(
                        a2f[:, :A2F], a1f[:, d3_off(t):d3_off(t) + A2F],
                        w3[:, t:t + 1], a2f[:, :A2F], OP.mult, OP.add)
                # stage D: 1x1 -> a3 slice, with per-tile sum accumulation
                for k in range(BLK * W // 512):
                    ps = pps.tile([128, 512], F32)
                    nc.tensor.matmul(ps[:], c1wB[:],
                                     a2[:, 2 * k:2 * k + 2, 5:261],
                                     start=True, stop=True)
                    col = acc_col[0]
                    acc_col[0] += 1
                    nc.scalar.activation(
                        a3dst[:, a3off + k * 512:a3off + (k + 1) * 512], ps[:],
                        AF.Identity, bias=b1, accum_out=accA[:, col:col + 1])

            # ---------------- conv phase ----------------
            for i in range(NKVB):
                do_block(ysl, i * BLK, C, s_kvwT, s_kv1wT, s_d5kv, s_d3kv,
                         s_w5kv, s_w3kv, s_bkv0, s_bkvs, s_bkv1,
                         i == 0, i == NKVB - 1, s_m0t_kv, s_m0b_kv,
                         a3kv, i * BLK * W)
            for i in range(NQB):
                do_block(xpk, i * BLK, 128, s_qwT2, s_q1wT2, s_d5q, s_d3q,
                         s_w5q, s_w3q, s_bq0, s_bqs, s_bq1,
                         i == 0, i == NQB - 1, s_m0t_q, s_m0b_q,
                         a3qp, i * BLK * W)

            # ---------------- sumsq passes ----------------
            junk = pbig.tile([128, SRC_R, WB], BF, tag="big")
            junkf = junk.rearrange("p r c -> p (r c)")
            CH = 4096
            nsq_kv = NKV // CH   # 8
            nsq_q = NQ // CH     # 4
            for k in range(nsq_kv):
                eng = nc.vector
                eng.scalar_tensor_tensor(
                    junkf[:, :CH], a3kv[:, k * CH:(k + 1) * CH], 1.0,
                    a3kv[:, k * CH:(k + 1) * CH], OP.mult, OP.mult,
                    accum_out=sqA[:, k:k + 1])
            for k in range(nsq_q):
                eng = nc.vector
                eng.scalar_tensor_tensor(
                    junkf[:, :CH], a3qp[:, k * CH:(k + 1) * CH], 1.0,
                    a3qp[:, k * CH:(k + 1) * CH], OP.mult, OP.mult,
                    accum_out=sqA[:, nsq_kv + k:nsq_kv + k + 1])

            # ---------------- stats pack + allreduce 1 ----------------
            stats = ptiny.tile([128, 4], F32, tag="stats")
            nkv_tiles = NKVB * BLK * W // 512
            nq_tiles = NQB * BLK * W // 512
            nc.vector.tensor_reduce(stats[:, 0:1], accA[:, 0:nkv_tiles],
                                    AX.X, OP.add)
            nc.vector.tensor_reduce(stats[:, 2:3],
                                    accA[:, nkv_tiles:nkv_tiles + nq_tiles],
                                    AX.X, OP.add)
            nc.vector.tensor_reduce(stats[:, 1:2], sqA[:, 0:nsq_kv],
                                    AX.X, OP.add)
            nc.vector.tensor_reduce(stats[:, 3:4],
                                    sqA[:, nsq_kv:nsq_kv + nsq_q],
                                    AX.X, OP.add)
            d_st = pdram.tile([128, 4], F32)
            d_str = pdram.tile([128, 4], F32)
            nc.gpsimd.dma_start(d_st[:], stats[:])
            nc.gpsimd.collective_compute(
                "AllReduce", OP.add,
                replica_groups=[[0, 1], [2, 3], [4, 5], [6, 7]],
                ins=[d_st.opt()], outs=[d_str.opt()])
            statsR = ptiny.tile([128, 4], F32, tag="statsR")
            nc.gpsimd.dma_start(statsR[:], d_str[:])

            # ---------------- group stats -> alpha/delta ----------------
            gps = ppsG.tile([4, 4], F32, tag="gpsum")
            nc.tensor.matmul(gps[:], s_ind[:], statsR[:], start=True, stop=True)
            gsb = ptiny.tile([4, 4], F32, tag="gsb")
            nc.vector.tensor_scalar(gsb[:], gps[:], s_cntr[:, 0:1], None,
                                    OP.mult)
            # cols: 0=kv mean,1=kv Ex2, 2=q mean,3=q Ex2
            mu = ptiny.tile([4, 2], F32, tag="mu")
            nc.vector.tensor_copy(mu[:, 0:1], gsb[:, 0:1])
            nc.vector.tensor_copy(mu[:, 1:2], gsb[:, 2:3])
            ex2 = ptiny.tile([4, 2], F32, tag="ex2")
            nc.vector.tensor_copy(ex2[:, 0:1], gsb[:, 1:2])
            nc.vector.tensor_copy(ex2[:, 1:2], gsb[:, 3:4])
            var = ptiny.tile([4, 2], F32, tag="var")
            nc.vector.tensor_mul(var[:], mu[:], mu[:])
            nc.vector.tensor_sub(var[:], ex2[:], var[:])
            nc.vector.tensor_scalar_add(var[:], var[:], GN_EPS)
            # rsqrt via reciprocal + sqrt + one NR step
            rv = ptiny.tile([4, 2], F32, tag="rv")
            nc.vector.reciprocal(rv[:], var[:])
            y0 = ptiny.tile([4, 2], F32, tag="y0")
            nc.scalar.sqrt(y0[:], rv[:])
            t0 = ptiny.tile([4, 2], F32, tag="t0")
            nc.vector.tensor_mul(t0[:], y0[:], y0[:])
            nc.vector.tensor_mul(t0[:], t0[:], var[:])
            nc.vector.tensor_scalar(t0[:], t0[:], -0.5, 1.5, OP.mult, OP.add)
            nc.vector.tensor_mul(y0[:], y0[:], t0[:])
            # broadcast group -> channels: [sg, mu] per chain
            gv_kv = ptiny.tile([4, 2], F32, tag="gvkv")
            nc.vector.tensor_copy(gv_kv[:, 0:1], y0[:, 0:1])
            nc.vector.tensor_copy(gv_kv[:, 1:2], mu[:, 0:1])
            gv_q = ptiny.tile([4, 2], F32, tag="gvq")
            nc.vector.tensor_copy(gv_q[:, 0:1], y0[:, 1:2])
            nc.vector.tensor_copy(gv_q[:, 1:2], mu[:, 1:2])

            def alpha_delta(bc, gv, gamma, beta, tag):
                bps = ppsG.tile([128, 2], F32, tag="gpsum")
                nc.tensor.matmul(bps[:], bc[:], gv[:], start=True, stop=True)
                pc = ptiny.tile([128, 2], F32, tag=tag + "pc")
                nc.vector.tensor_copy(pc[:], bps[:])
                al = ptiny.tile([128, 1], F32, tag=tag + "al")
                nc.vector.tensor_mul(al[:], pc[:, 0:1], gamma[:])
                de = ptiny.tile([128, 1], F32, tag=tag + "de")
                nc.vector.tensor_mul(de[:], pc[:, 1:2], al[:])
                nc.vector.tensor_sub(de[:], beta[:], de[:])
                return al, de

            al_kv, de_kv = alpha_delta(s_bckv, gv_kv, s_gkv, s_bekv, "kv")
            al_q, de_q = alpha_delta(s_bcq, gv_q, s_gq, s_beq, "q")

            # ---------------- u-pass (GN affine + leaky relu) ----------
            nc.scalar.activation(a3kv[:], a3kv[:], AF.Identity,
                                 bias=de_kv[:], scale=al_kv[:])
            nc.scalar.activation(a3qp[:], a3qp[:], AF.Identity,
                                 bias=de_q[:], scale=al_q[:])
            for k in range(2):
                h = NKV // 2
                nc.vector.scalar_tensor_tensor(
                    a3kv[:, k * h:(k + 1) * h], a3kv[:, k * h:(k + 1) * h],
                    0.2, a3kv[:, k * h:(k + 1) * h], OP.mult, OP.max)
            nc.vector.scalar_tensor_tensor(
                a3qp[:], a3qp[:], 0.2, a3qp[:], OP.mult, OP.max)

            # ---------------- norms (sumsq of u) ----------------------
            qn2 = pers.tile([128, 4], F32)
            kn2 = pers.tile([64, 8], F32)
            for k in range(4):
                nc.vector.scalar_tensor_tensor(
                    junkf[:, :CH], a3qp[:, k * CH:(k + 1) * CH], 1.0,
                    a3qp[:, k * CH:(k + 1) * CH], OP.mult, OP.mult,
                    accum_out=qn2[:, k:k + 1])
            for k in range(8):
                nc.vector.scalar_tensor_tensor(
                    junkf[:64, :CH], a3kv[:64, k * CH:(k + 1) * CH], 1.0,
                    a3kv[:64, k * CH:(k + 1) * CH], OP.mult, OP.mult,
                    accum_out=kn2[:, k:k + 1])

            # ---------------- gram phase: G_qk ----------------
            def _cp(eng, dst, srcap):
                if eng is nc.scalar:
                    eng.copy(dst, srcap)
                else:
                    eng.tensor_copy(dst, srcap)

            Gq = ppsG.tile([64, 64], F32, tag="gpsum")
            NCH = NQ // 128  # 128 q chunks
            for i in range(NCH):
                tps = ppsT.tile([128, 128], BF, tag="tps")
                nc.tensor.transpose(tps[:], a3qp[:, i * 128:(i + 1) * 128],
                                    s_idn[:])
                tq = ptchk.tile([128, 128], BF, tag="tq")
                (nc.vector if i % 2 == 0 else nc.scalar).tensor_copy(
                    tq[:], tps[:]) if False else None
                _cp([nc.vector, nc.scalar][i % 2], tq[:], tps[:])
                tps0 = ppsT.tile([128, 128], BF, tag="tps")
                nc.tensor.transpose(tps0[:, :64],
                                    a3kv[:64, i * 128:(i + 1) * 128],
                                    s_idn[:64, :64])
                tk0 = ptchk.tile([128, 64], BF, tag="tk0")
                _cp([nc.scalar, nc.vector][i % 2], tk0[:], tps0[:, :64])
                tps1 = ppsT.tile([128, 128], BF, tag="tps")
                nc.tensor.transpose(
                    tps1[:, :64],
                    a3kv[:64, NQ + i * 128:NQ + (i + 1) * 128],
                    s_idn[:64, :64])
                tk1 = ptchk.tile([128, 64], BF, tag="tk1")
                _cp([nc.vector, nc.scalar][(i + 1) % 2], tk1[:], tps1[:, :64])
                nc.tensor.matmul(Gq[:], tq[:, 0:64], tk0[:],
                                 start=(i == 0), stop=False,
                                 skip_group_check=True)
                nc.tensor.matmul(Gq[:], tq[:, 64:128], tk1[:],
                                 start=False, stop=(i == NCH - 1),
                                 skip_group_check=True)

            # ---------------- pack + allreduce 2 ----------------
            nc.gpsimd.memset(av2[:], 0.0)
            nc.vector.tensor_copy(av2[:64, 0:64], Gq[:])
            nc.vector.tensor_reduce(av2[:, 64:65], qn2[:], AX.X, OP.add)
            nc.vector.tensor_reduce(av2[:64, 65:66], kn2[:], AX.X, OP.add)
            d_av = pdram.tile([128, 66], F32)
            d_avr = pdram.tile([128, 66], F32)
            nc.gpsimd.dma_start(d_av[:], av2[:])
            nc.gpsimd.collective_compute(
                "AllReduce", OP.add,
                replica_groups=[[0, 1], [2, 3], [4, 5], [6, 7]],
                ins=[d_av.opt()], outs=[d_avr.opt()])
            avr = pers.tile([128, 66], F32)
            nc.gpsimd.dma_start(avr[:], d_avr[:])

            # ---------------- tiny attention ----------------
            qtmp = ptiny.tile([64, 1], F32, tag="qtmp")
            nc.sync.dma_start(qtmp[:], avr[64:128, 64:65])
            nrm2 = ptiny.tile([64, 2], F32, tag="nrm2")
            nc.vector.tensor_add(nrm2[:, 0:1], avr[:64, 64:65], qtmp[:])
            nc.vector.tensor_copy(nrm2[:, 1:2], avr[:64, 65:66])
            rn = ptiny.tile([64, 2], F32, tag="rn")
            nc.vector.reciprocal(rn[:], nrm2[:])
            yn = ptiny.tile([64, 2], F32, tag="yn")
            nc.scalar.sqrt(yn[:], rn[:])
            tn = ptiny.tile([64, 2], F32, tag="tn")
            nc.vector.tensor_mul(tn[:], yn[:], yn[:])
            nc.vector.tensor_mul(tn[:], tn[:], nrm2[:])
            nc.vector.tensor_scalar(tn[:], tn[:], -0.5, 1.5, OP.mult, OP.add)
            nc.vector.tensor_mul(yn[:], yn[:], tn[:])
            rq = ptiny.tile([64, 1], F32, tag="rq")
            nc.vector.tensor_mul(rq[:], yn[:, 0:1], s_tau[:])
            # rk broadcast across free dim
            rkT = ppsG.tile([1, 64], F32, tag="gpsum")
            nc.tensor.transpose(rkT[:], yn[:, 1:2], s_idnf[:])
            rkrow = ptiny.tile([1, 64], F32, tag="rkrow")
            nc.vector.tensor_copy(rkrow[:], rkT[:])
            rkbc = ptiny.tile([64, 64], F32, tag="rkbc")
            nc.gpsimd.partition_broadcast(rkbc[:], rkrow[:])
            # logits
            L = ptiny.tile([64, 64], F32, tag="L")
            nc.vector.tensor_copy(L[:], avr[:64, 0:64])
            nc.vector.tensor_scalar_mul(L[:], L[:], rq[:])
            nc.vector.tensor_mul(L[:], L[:], rkbc[:])
            nc.scalar.activation(L[:], L[:], AF.Exp)
            nc.vector.tensor_mul(L[:], L[:], s_bmask[:])
            rs = ptiny.tile([64, 1], F32, tag="rs")
            nc.vector.tensor_reduce(rs[:], L[:], AX.X, OP.add)
            nc.vector.reciprocal(rs[:], rs[:])
            nc.vector.tensor_scalar_mul(L[:], L[:], rs[:])
            # W2 = Abd^T @ P^T  -> [vc, o]
            w2ps = ppsG.tile([64, 64], F32, tag="gpsum")
            nc.tensor.matmul(w2ps[:], L[:], s_projT[:], start=True, stop=True)
            w2sb = ptiny.tile([64, 64], BF, tag="w2sb")
            nc.scalar.copy(w2sb[:], w2ps[:])
            W2big = pers.tile([128, 64], BF)
            nc.gpsimd.memset(W2big[:64, :], 0.0)
            nc.sync.dma_start(W2big[64:128, :], w2sb[:])

            # ---------------- out = (P@Abd) @ v ----------------
            for k in range(NKV // 512):
                ps = pps.tile([64, 512], F32)
                nc.tensor.matmul(ps[:], W2big[:],
                                 a3kv[:, k * 512:(k + 1) * 512],
                                 start=True, stop=True)
                osb = posb.tile([64, 512], F32, tag="osb")
                _cp(nc.scalar, osb[:], ps[:])
                nc.sync.dma_start(out_d[:, k * 512:(k + 1) * 512], osb[:])

    nc.compile()
    _CACHE["nc"] = nc
    return nc


def _prep(inputs):
    bf16 = ml_dtypes.bfloat16
    x = np.asarray(inputs["x"], np.float32)
    y = np.asarray(inputs["y"], np.float32)

    def z(*s):
        return np.zeros(s, np.float32)

    # weights (shared across cores)
    kv_w = np.asarray(inputs["kv_w"], np.float32)[:, :, 0, 0]
    q_w = np.asarray(inputs["q_w"], np.float32)[:, :, 0, 0]
    proj_w = np.asarray(inputs["proj_w"], np.float32)[:, :, 0, 0]
    kv1 = np.asarray(inputs["kv_c1_w"], np.float32)[:, :, 0, 0]
    q1 = np.asarray(inputs["q_c1_w"], np.float32)[:, :, 0, 0]

    def blockdiag(a):
        o = z(128, 128)
        o[:64, :64] = a
        o[64:, 64:] = a
        return o

    w5kv_ = np.asarray(inputs["kv_c0_w"], np.float32)[:, 0].reshape(128, 25)
    w3kv_ = np.asarray(inputs["kv_cs_w"], np.float32)[:, 0].reshape(128, 9)
    w5q1 = np.asarray(inputs["q_c0_w"], np.float32)[:, 0].reshape(64, 25)
    w3q1 = np.asarray(inputs["q_cs_w"], np.float32)[:, 0].reshape(64, 9)
    w5q_ = np.concatenate([w5q1, w5q1], 0)
    w3q_ = np.concatenate([w3q1, w3q1], 0)

    def diags(wv, taps):
        o = z(128, len(taps) * 128)
        for j, t in enumerate(taps):
            o[np.arange(128), j * 128 + np.arange(128)] = wv[:, t]
        return o

    def dup(v):
        return np.concatenate([v, v], 0).reshape(128, 1)

    com = {
        "kvwT": kv_w.T.astype(bf16),
        "kv1wT": kv1.T.astype(bf16),
        "qwT2": blockdiag(q_w.T).astype(bf16),
        "q1wT2": blockdiag(q1.T).astype(bf16),
        "d5kv": diags(w5kv_, PE5).astype(bf16),
        "d3kv": diags(w3kv_, PE3).astype(bf16),
        "d5q": diags(w5q_, PE5).astype(bf16),
        "d3q": diags(w3q_, PE3).astype(bf16),
        "w5kv": w5kv_, "w3kv": w3kv_, "w5q": w5q_, "w3q": w3q_,
        "bkv0": np.asarray(inputs["kv_c0_b"], np.float32).reshape(128, 1),
        "bkvs": np.asarray(inputs["kv_cs_b"], np.float32).reshape(128, 1),
        "bkv1": np.asarray(inputs["kv_c1_b"], np.float32).reshape(128, 1),
        "bq0": dup(np.asarray(inputs["q_c0_b"], np.float32)),
        "bqs": dup(np.asarray(inputs["q_cs_b"], np.float32)),
        "bq1": dup(np.asarray(inputs["q_c1_b"], np.float32)),
        "g_kv": np.asarray(inputs["kv_gn_g"], np.float32).reshape(128, 1),
        "be_kv": np.asarray(inputs["kv_gn_b"], np.float32).reshape(128, 1),
        "g_q": dup(np.asarray(inputs["q_gn_g"], np.float32)),
        "be_q": dup(np.asarray(inputs["q_gn_b"], np.float32)),
        "tau64": np.repeat(np.asarray(inputs["temperature"],
                                      np.float32).reshape(4), 16).reshape(64, 1),
        "projT": proj_w.T.copy(),
        "idn": np.eye(128, dtype=np.float32).astype(bf16),
        "idnf": np.eye(64, dtype=np.float32),
    }
    ind = z(128, 4)
    ind[0:64, 0] = 1.0
    ind[64:128, 1] = 1.0
    pp = np.arange(128) % 64
    ind[pp < 32, 2] = 1.0
    ind[pp >= 32, 3] = 1.0
    com["ind"] = ind
    bckv = z(4, 128)
    bckv[0, 0:64] = 1.0
    bckv[1, 64:128] = 1.0
    com["bc_kv"] = bckv
    bcq = z(4, 128)
    bcq[2, pp < 32] = 1.0
    bcq[3, pp >= 32] = 1.0
    com["bc_q"] = bcq
    com["cntr"] = np.array([[1.0 / (64 * H * W)], [1.0 / (64 * H * W)],
                            [1.0 / (32 * H * W)], [1.0 / (32 * H * W)]],
                           np.float32)
    bm = z(64, 64)
    for h in range(4):
        bm[h * 16:(h + 1) * 16, h * 16:(h + 1) * 16] = 1.0
    com["bmask"] = bm

    in_maps = []
    for core in range(N_CORES):
        b, half = core // 2, core % 2
        r0 = half * R
        ysl = z(C, R + 10, WB)
        lo, hi = r0 - 5, r0 + R + 5
        slo, shi = max(lo, 0), min(hi, H)
        ysl[:, slo - lo:shi - lo, 5:261] = y[b, :, slo:shi, :]
        xpk = z(128, R // 2 + 10, WB)
        for hf in range(2):
            base = r0 + hf * (R // 2)
            lo2, hi2 = base - 5, base + R // 2 + 5
            s2, e2 = max(lo2, 0), min(hi2, H)
            xpk[hf * 64:(hf + 1) * 64, s2 - lo2:e2 - lo2, 5:261] = \
                x[b, :, s2:e2, :]
        m = dict(com)
        m["ysl"] = ysl.reshape(C, -1).astype(bf16)
        m["xpk"] = xpk.reshape(128, -1).astype(bf16)
        m["m0t_kv"] = np.full((128, 1), 0.0 if r0 == 0 else 1.0, np.float32)
        m["m0b_kv"] = np.full((128, 1), 0.0 if r0 + R == H else 1.0,
                              np.float32)
        mtq = np.ones((128, 1), np.float32)
        if r0 == 0:
            mtq[0:64] = 0.0
        m["m0t_q"] = mtq
        mbq = np.ones((128, 1), np.float32)
        if r0 + R == H:
            mbq[64:128] = 0.0
        m["m0b_q"] = mbq
        in_maps.append(m)
    return in_maps


def kernel(**inputs):
    import os
    from concourse.bass_utils import run_bass_kernel_spmd

    nc = _build()
    in_maps = _prep(inputs)
    trace = bool(os.environ.get("BASS_KERNEL_TRACE"))
    res = run_bass_kernel_spmd(nc, in_maps, list(range(N_CORES)),
                               trace=trace)
    global _LAST_EXEC_NS
    _LAST_EXEC_NS = res.exec_time_ns
    import kernel as _self
    _self._LAST_EXEC_NS = res.exec_time_ns
    _CACHE["res"] = res
    out = np.zeros((B, C, H, W), np.float32)
    for core in range(N_CORES):
        b, half = core // 2, core % 2
        out[b, :, half * R:(half + 1) * R, :] = \
            res.results[core]["out"].reshape(C, R, W)
    return out



# revision 7
# speedup vs baseline: 4.0803x; 4.0803x over previous
import sys

sys.path.insert(0, "/opt/trn_rl_repo")

import numpy as np
import ml_dtypes

# ---------------- constants (hardcoded problem geometry) ----------------
B, C, H, W = 4, 64, 256, 256
HEADS = 4
N_CORES = 8
R = 128             # sample rows per core (H split in 2)
WB = W + 10         # padded width 266
BLK = 16            # output rows per block
NKVB = R // BLK     # 8 kv blocks
NQB = (R // 2) // BLK  # 4 q blocks (packed halves)
SRC_R = BLK + 10    # 26 src/a0 rows per block
A1_R = BLK + 6      # 22 a1 content rows
A0F = SRC_R * WB    # 6916
A1F = A1_R * WB     # 5852
A2F = BLK * WB      # 4256
NKV = R * W         # 32768
NQ = (R // 2) * W   # 16384
GN_EPS = 1e-5

# ---- blob layout (single int8 ExternalInput per core, [128, BPP]) ----
# xa: packed x halves, [128, 74*256] int8
OXA = 0
XA_B = 74 * W                       # 18944
# ya: y rows split across partition halves:
#   partitions 0:64   hold channel p rows 0:69   of the 138-row halo space
#   partitions 64:128 hold channel p-64 rows 69:138
OYA = OXA + XA_B
YSPLIT = 69
YA_B = YSPLIT * W                   # 17664
OWT = OYA + YA_B                    # weights region start (36608)


def d5_off(t):
    return (t // 5) * WB + (t % 5)


def d3_off(t):
    # a1 column basis: data col = j - 3  ->  col offset 3*kw - 5
    return WB + (t // 3) * 3 * WB + ((t % 3) * 3 - 5)


# tap assignment: DVE keeps only 4B-aligned (even-offset) taps for 2x mode;
# PE takes all odd-offset taps plus extra even ones for engine balance.
_odd5 = [t for t in range(25) if (t % 5) in (1, 3)]
_ev5 = [t for t in range(25) if (t % 5) in (0, 2, 4)]
PE5 = _odd5 + [_ev5[0], _ev5[4], _ev5[10], _ev5[14]]         # 14
DVE5 = [t for t in _ev5 if t not in PE5]                     # 11
PE3 = [0, 2, 3, 5, 6, 8]   # odd-offset taps (kw!=1) + balance
DVE3 = [1, 4, 7]           # kw==1 -> even offset -> 2x eligible

# weight sub-layout inside the blob: (name, partitions, bytes-per-partition)
_WSPEC = [
    ("kvwT", 64, 256),    # bf16 [64,128], pre-scaled by 1/sy
    ("kv1wT", 128, 256),  # bf16 [128,128]
    ("qwT2", 128, 256),   # bf16 [128,128], pre-scaled by 1/sx
    ("q1wT2", 128, 256),  # bf16 [128,128]
    ("w5kv", 128, 104),   # f32 [128,25] (+pad)
    ("w3kv", 128, 40),    # f32 [128,9] (+pad)
    ("w5q", 128, 104),
    ("w3q", 128, 40),
    ("bkv0", 128, 4), ("bkvs", 128, 4), ("bkv1", 128, 4),
    ("bq0", 128, 4), ("bqs", 128, 4), ("bq1", 128, 4),
    ("m0t_kv", 128, 4), ("m0b_kv", 128, 4),
    ("m0t_q", 128, 4), ("m0b_q", 128, 4),
    ("g_kv", 128, 4), ("be_kv", 128, 4),
    ("g_q", 128, 4), ("be_q", 128, 4),
    ("ind", 128, 16),
    ("bc_kv", 4, 512), ("bc_q", 4, 512),
    ("cntr", 4, 4), ("tau64", 64, 4), ("bmask", 64, 256),
    ("idn", 128, 256),    # bf16 identity
    ("idnf", 64, 256),    # f32 identity 64
    ("projT", 64, 256),   # f32 [64,64]
]
_WOFF = {}
_off = OWT
for _n, _p, _b in _WSPEC:
    _WOFF[_n] = _off
    _off += _b
BPP = (_off + 63) // 64 * 64         # pad to 64B

_CACHE = {}


def _build():
    if "nc" in _CACHE:
        return _CACHE["nc"]
    import concourse.bacc as bacc
    import concourse.tile as tile
    from concourse import mybir

    BF = mybir.dt.bfloat16
    F32 = mybir.dt.float32
    I8 = mybir.dt.int8
    AF = mybir.ActivationFunctionType
    OP = mybir.AluOpType
    AX = mybir.AxisListType

    nc = bacc.Bacc("TRN2", target_bir_lowering=False, debug=False,
                   num_devices=N_CORES)

    blob = nc.dram_tensor("blob", [128, BPP], I8, kind="ExternalInput").ap()
    out_d = nc.dram_tensor("out", [C, NKV], BF, kind="ExternalOutput").ap()

    def wslice(name, dt):
        p, nb = next((pp, bb) for nn, pp, bb in _WSPEC if nn == name)
        return blob[:p, _WOFF[name]:_WOFF[name] + nb].bitcast(dt)

    def ceil(a, b):
        return (a + b - 1) // b

    with tile.TileContext(nc) as tc:
        with (
            tc.tile_pool(name="big", bufs=3) as pbig,
            tc.tile_pool(name="s8", bufs=2) as ps8,
            tc.tile_pool(name="a1p", bufs=2) as pa1,
            tc.tile_pool(name="pers", bufs=1) as pers,
            tc.tile_pool(name="wts", bufs=1) as pwts,
            tc.tile_pool(name="tiny", bufs=1) as ptiny,
            tc.tile_pool(name="tchk", bufs=4) as ptchk,
            tc.tile_pool(name="osbp", bufs=2) as posb,
            tc.tile_pool(name="ps", bufs=4, space="PSUM") as pps,
            tc.tile_pool(name="psT", bufs=3, space="PSUM") as ppsT,
            tc.tile_pool(name="psG", bufs=1, space="PSUM") as ppsG,
            tc.tile_pool(name="dram", bufs=1, space="DRAM") as pdram,
        ):
            a3kv = pers.tile([128, NKV], BF)
            a3qp = pers.tile([128, NQ], BF)
            accA = pers.tile([128, 96], F32)
            sqA = pers.tile([128, 12], F32)
            av2 = pers.tile([128, 66], F32)

            def wtile(name, shape, dt):
                t = pwts.tile(list(shape), dt, tag="w_" + name)
                nc.sync.dma_start(out=t[:], in_=wslice(name, dt))
                return t

            s_kvwT = wtile("kvwT", [64, 128], BF)
            s_kv1wT = wtile("kv1wT", [128, 128], BF)
            s_qwT2 = wtile("qwT2", [128, 128], BF)
            s_q1wT2 = wtile("q1wT2", [128, 128], BF)
            s_w5kv = wtile("w5kv", [128, 26], F32)
            s_w3kv = wtile("w3kv", [128, 10], F32)
            s_w5q = wtile("w5q", [128, 26], F32)
            s_w3q = wtile("w3q", [128, 10], F32)
            s_bkv0 = wtile("bkv0", [128, 1], F32)
            s_bkvs = wtile("bkvs", [128, 1], F32)
            s_bkv1 = wtile("bkv1", [128, 1], F32)
            s_bq0 = wtile("bq0", [128, 1], F32)
            s_bqs = wtile("bqs", [128, 1], F32)
            s_bq1 = wtile("bq1", [128, 1], F32)
            s_m0t_kv = wtile("m0t_kv", [128, 1], F32)
            s_m0b_kv = wtile("m0b_kv", [128, 1], F32)
            s_m0t_q = wtile("m0t_q", [128, 1], F32)
            s_m0b_q = wtile("m0b_q", [128, 1], F32)
            s_gkv = wtile("g_kv", [128, 1], F32)
            s_bekv = wtile("be_kv", [128, 1], F32)
            s_gq = wtile("g_q", [128, 1], F32)
            s_beq = wtile("be_q", [128, 1], F32)
            s_ind = wtile("ind", [128, 4], F32)
            s_bckv = wtile("bc_kv", [4, 128], F32)
            s_bcq = wtile("bc_q", [4, 128], F32)
            s_cntr = wtile("cntr", [4, 1], F32)
            s_tau = wtile("tau64", [64, 1], F32)
            s_bmask = wtile("bmask", [64, 64], F32)
            s_idn = wtile("idn", [128, 128], BF)
            s_idnf = wtile("idnf", [64, 64], F32)
            s_projT = wtile("projT", [64, 64], F32)

            # build diagonal tap matrices on device: diag(w[:, t]) per tap
            def build_diag(wv, taps, tag):
                t = pwts.tile([128, len(taps) * 128], BF, tag=tag)
                for j, tp in enumerate(taps):
                    nc.vector.tensor_scalar_mul(
                        t[:, j * 128:(j + 1) * 128], s_idn[:],
                        wv[:, tp:tp + 1])
                return t

            s_d5kv = build_diag(s_w5kv, PE5, "d5kv")
            s_d3kv = build_diag(s_w3kv, PE3, "d3kv")
            s_d5q = build_diag(s_w5q, PE5, "d5q")
            s_d3q = build_diag(s_w3q, PE3, "d3q")

            acc_col = [0]

            def load_src_kv(i):
                # int8 tile of 26 rows x 256 cols from the split ya region
                a = i * BLK
                s8 = ps8.tile([128, SRC_R, W], I8, tag="s8")
                n1 = min(SRC_R, max(0, YSPLIT - a))
                if n1 > 0:
                    nc.sync.dma_start(
                        out=s8[:64, 0:n1, :],
                        in_=blob[0:64, OYA + a * W:OYA + (a + n1) * W]
                        .rearrange("p (r c) -> p r c", c=W))
                if n1 < SRC_R:
                    a2 = max(a, YSPLIT) - YSPLIT
                    n2 = SRC_R - n1
                    nc.sync.dma_start(
                        out=s8[:64, n1:SRC_R, :],
                        in_=blob[64:128, OYA + a2 * W:OYA + (a2 + n2) * W]
                        .rearrange("p (r c) -> p r c", c=W))
                return s8

            def load_src_q(i):
                a = i * BLK
                s8 = ps8.tile([128, SRC_R, W], I8, tag="s8")
                nc.sync.dma_start(
                    out=s8[:, :, :],
                    in_=blob[:, OXA + a * W:OXA + (a + SRC_R) * W]
                    .rearrange("p (r c) -> p r c", c=W))
                return s8

            def do_block(load_src, K, c1wA, c1wB, d5, d3, w5, w3,
                         b0, bs, b1, first, last, mt, mb, a3dst, a3off, i):
                s8 = load_src(i)
                src = pbig.tile([128, SRC_R, WB], BF, tag="big")
                nc.gpsimd.memset(src[:K, :, 0:5], 0.0)
                nc.gpsimd.memset(src[:K, :, 261:266], 0.0)
                nc.scalar.copy(src[:K, :, 5:261], s8[:K])
                srcf = src.rearrange("p r c -> p (r c)")
                # stage A: conv1x1 -> a0
                a0 = pbig.tile([128, A0F + 16], BF, tag="big")
                a0f = a0
                for k in range(ceil(A0F, 512)):
                    n = min(512, A0F - k * 512)
                    ps = pps.tile([128, 512], F32)
                    nc.tensor.matmul(ps[:, :n], c1wA[:K],
                                     srcf[:K, k * 512:k * 512 + n],
                                     start=True, stop=True)
                    nc.scalar.copy(a0f[:, k * 512:k * 512 + n], ps[:, :n])
                # stage B: dw5x5 -> a1
                a1 = pa1.tile([128, A1_R + 2, WB], BF, tag="a1")
                a1f = a1.rearrange("p r c -> p (r c)")
                a1c = a1f[:, WB:WB + A1F]
                for k in range(ceil(A1F, 512)):
                    n = min(512, A1F - k * 512)
                    ps = pps.tile([128, 512], F32)
                    for j, t in enumerate(PE5):
                        nc.tensor.matmul(
                            ps[:, :n], d5[:, j * 128:(j + 1) * 128],
                            a0f[:, k * 512 + d5_off(t):k * 512 + d5_off(t) + n],
                            start=(j == 0), stop=(j == len(PE5) - 1))
                    nc.scalar.activation(a1f[:, WB + k * 512:WB + k * 512 + n],
                                         ps[:, :n], AF.Identity, bias=b0)
                for t in DVE5:
                    nc.vector.scalar_tensor_tensor(
                        a1c, a0f[:, d5_off(t):d5_off(t) + A1F], w5[:, t:t + 1],
                        a1c, OP.mult, OP.add)
                if first:
                    nc.vector.tensor_scalar_mul(a1f[:, WB:WB + 3 * WB],
                                                a1f[:, WB:WB + 3 * WB], mt)
                if last:
                    lo = WB + (A1_R - 3) * WB
                    nc.vector.tensor_scalar_mul(a1f[:, lo:lo + 3 * WB],
                                                a1f[:, lo:lo + 3 * WB], mb)
                nc.gpsimd.memset(a1[:, 1:, 0:3], 0.0)
                nc.gpsimd.memset(a1[:, 1:, 259:266], 0.0)
                # stage C: dw3x3 dil3 -> a2
                a2 = pbig.tile([128, SRC_R, WB], BF, tag="big")
                a2f = a2.rearrange("p r c -> p (r c)")
                for k in range(ceil(A2F, 512)):
                    n = min(512, A2F - k * 512)
                    ps = pps.tile([128, 512], F32)
                    for j, t in enumerate(PE3):
                        nc.tensor.matmul(
                            ps[:, :n], d3[:, j * 128:(j + 1) * 128],
                            a1f[:, k * 512 + d3_off(t):k * 512 + d3_off(t) + n],
                            start=(j == 0), stop=(j == len(PE3) - 1))
                    nc.scalar.activation(a2f[:, k * 512:k * 512 + n],
                                         ps[:, :n], AF.Identity, bias=bs)
                for t in DVE3:
                    nc.vector.scalar_tensor_tensor(
                        a2f[:, :A2F], a1f[:, d3_off(t):d3_off(t) + A2F],
                        w3[:, t:t + 1], a2f[:, :A2F], OP.mult, OP.add)
                # stage D: 1x1 -> a3 slice, with per-tile sum accumulation
                for k in range(BLK * W // 512):
                    ps = pps.tile([128, 512], F32)
                    nc.tensor.matmul(ps[:], c1wB[:],
                                     a2[:, 2 * k:2 * k + 2, 5:261],
                                     start=True, stop=True)
                    col = acc_col[0]
                    acc_col[0] += 1
                    nc.scalar.activation(
                        a3dst[:, a3off + k * 512:a3off + (k + 1) * 512], ps[:],
                        AF.Identity, bias=b1, accum_out=accA[:, col:col + 1])

            # ---------------- conv phase ----------------
            for i in range(NKVB):
                do_block(load_src_kv, C, s_kvwT, s_kv1wT, s_d5kv, s_d3kv,
                         s_w5kv, s_w3kv, s_bkv0, s_bkvs, s_bkv1,
                         i == 0, i == NKVB - 1, s_m0t_kv, s_m0b_kv,
                         a3kv, i * BLK * W, i)
            for i in range(NQB):
                do_block(load_src_q, 128, s_qwT2, s_q1wT2, s_d5q, s_d3q,
                         s_w5q, s_w3q, s_bq0, s_bqs, s_bq1,
                         i == 0, i == NQB - 1, s_m0t_q, s_m0b_q,
                         a3qp, i * BLK * W, i)

            # ---------------- sumsq passes ----------------
            junk = pbig.tile([128, SRC_R, WB], BF, tag="big")
            junkf = junk.rearrange("p r c -> p (r c)")
            CH = 4096
            nsq_kv = NKV // CH   # 8
            nsq_q = NQ // CH     # 4
            for k in range(nsq_kv):
                nc.vector.scalar_tensor_tensor(
                    junkf[:, :CH], a3kv[:, k * CH:(k + 1) * CH], 1.0,
                    a3kv[:, k * CH:(k + 1) * CH], OP.mult, OP.mult,
                    accum_out=sqA[:, k:k + 1])
            for k in range(nsq_q):
                nc.vector.scalar_tensor_tensor(
                    junkf[:, :CH], a3qp[:, k * CH:(k + 1) * CH], 1.0,
                    a3qp[:, k * CH:(k + 1) * CH], OP.mult, OP.mult,
                    accum_out=sqA[:, nsq_kv + k:nsq_kv + k + 1])

            # ---------------- stats pack + allreduce 1 ----------------
            stats = ptiny.tile([128, 4], F32, tag="stats")
            nkv_tiles = NKVB * BLK * W // 512
            nq_tiles = NQB * BLK * W // 512
            nc.vector.tensor_reduce(stats[:, 0:1], accA[:, 0:nkv_tiles],
                                    AX.X, OP.add)
            nc.vector.tensor_reduce(stats[:, 2:3],
                                    accA[:, nkv_tiles:nkv_tiles + nq_tiles],
                                    AX.X, OP.add)
            nc.vector.tensor_reduce(stats[:, 1:2], sqA[:, 0:nsq_kv],
                                    AX.X, OP.add)
            nc.vector.tensor_reduce(stats[:, 3:4],
                                    sqA[:, nsq_kv:nsq_kv + nsq_q],
                                    AX.X, OP.add)
            d_st = pdram.tile([128, 4], F32)
            d_str = pdram.tile([128, 4], F32)
            nc.gpsimd.dma_start(d_st[:], stats[:])
            nc.gpsimd.collective_compute(
                "AllReduce", OP.add,
                replica_groups=[[0, 1], [2, 3], [4, 5], [6, 7]],
                ins=[d_st.opt()], outs=[d_str.opt()])
            statsR = ptiny.tile([128, 4], F32, tag="statsR")
            nc.gpsimd.dma_start(statsR[:], d_str[:])

            # ---------------- group stats -> alpha/delta ----------------
            gps = ppsG.tile([4, 4], F32, tag="gpsum")
            nc.tensor.matmul(gps[:], s_ind[:], statsR[:], start=True, stop=True)
            gsb = ptiny.tile([4, 4], F32, tag="gsb")
            nc.vector.tensor_scalar(gsb[:], gps[:], s_cntr[:, 0:1], None,
                                    OP.mult)
            # cols: 0=kv mean,1=kv Ex2, 2=q mean,3=q Ex2
            mu = ptiny.tile([4, 2], F32, tag="mu")
            nc.vector.tensor_copy(mu[:, 0:1], gsb[:, 0:1])
            nc.vector.tensor_copy(mu[:, 1:2], gsb[:, 2:3])
            ex2 = ptiny.tile([4, 2], F32, tag="ex2")
            nc.vector.tensor_copy(ex2[:, 0:1], gsb[:, 1:2])
            nc.vector.tensor_copy(ex2[:, 1:2], gsb[:, 3:4])
            var = ptiny.tile([4, 2], F32, tag="var")
            nc.vector.tensor_mul(var[:], mu[:], mu[:])
            nc.vector.tensor_sub(var[:], ex2[:], var[:])
            nc.vector.tensor_scalar_add(var[:], var[:], GN_EPS)
            # rsqrt via reciprocal + sqrt + one NR step
            rv = ptiny.tile([4, 2], F32, tag="rv")
            nc.vector.reciprocal(rv[:], var[:])
            y0 = ptiny.tile([4, 2], F32, tag="y0")
            nc.scalar.sqrt(y0[:], rv[:])
            t0 = ptiny.tile([4, 2], F32, tag="t0")
            nc.vector.tensor_mul(t0[:], y0[:], y0[:])
            nc.vector.tensor_mul(t0[:], t0[:], var[:])
            nc.vector.tensor_scalar(t0[:], t0[:], -0.5, 1.5, OP.mult, OP.add)
            nc.vector.tensor_mul(y0[:], y0[:], t0[:])
            # broadcast group -> channels: [sg, mu] per chain
            gv_kv = ptiny.tile([4, 2], F32, tag="gvkv")
            nc.vector.tensor_copy(gv_kv[:, 0:1], y0[:, 0:1])
            nc.vector.tensor_copy(gv_kv[:, 1:2], mu[:, 0:1])
            gv_q = ptiny.tile([4, 2], F32, tag="gvq")
            nc.vector.tensor_copy(gv_q[:, 0:1], y0[:, 1:2])
            nc.vector.tensor_copy(gv_q[:, 1:2], mu[:, 1:2])

            def alpha_delta(bc, gv, gamma, beta, tag):
                bps = ppsG.tile([128, 2], F32, tag="gpsum")
                nc.tensor.matmul(bps[:], bc[:], gv[:], start=True, stop=True)
                pc = ptiny.tile([128, 2], F32, tag=tag + "pc")
                nc.vector.tensor_copy(pc[:], bps[:])
                al = ptiny.tile([128, 1], F32, tag=tag + "al")
                nc.vector.tensor_mul(al[:], pc[:, 0:1], gamma[:])
                de = ptiny.tile([128, 1], F32, tag=tag + "de")
                nc.vector.tensor_mul(de[:], pc[:, 1:2], al[:])
                nc.vector.tensor_sub(de[:], beta[:], de[:])
                return al, de

            al_kv, de_kv = alpha_delta(s_bckv, gv_kv, s_gkv, s_bekv, "kv")
            al_q, de_q = alpha_delta(s_bcq, gv_q, s_gq, s_beq, "q")

            # ---------------- u-pass (GN affine + leaky relu) ----------
            nc.scalar.activation(a3kv[:], a3kv[:], AF.Identity,
                                 bias=de_kv[:], scale=al_kv[:])
            nc.scalar.activation(a3qp[:], a3qp[:], AF.Identity,
                                 bias=de_q[:], scale=al_q[:])
            for k in range(2):
                h = NKV // 2
                nc.vector.scalar_tensor_tensor(
                    a3kv[:, k * h:(k + 1) * h], a3kv[:, k * h:(k + 1) * h],
                    0.2, a3kv[:, k * h:(k + 1) * h], OP.mult, OP.max)
            nc.vector.scalar_tensor_tensor(
                a3qp[:], a3qp[:], 0.2, a3qp[:], OP.mult, OP.max)

            # ---------------- norms (sumsq of u) ----------------------
            qn2 = pers.tile([128, 4], F32)
            kn2 = pers.tile([64, 8], F32)
            for k in range(4):
                nc.vector.scalar_tensor_tensor(
                    junkf[:, :CH], a3qp[:, k * CH:(k + 1) * CH], 1.0,
                    a3qp[:, k * CH:(k + 1) * CH], OP.mult, OP.mult,
                    accum_out=qn2[:, k:k + 1])
            for k in range(8):
                nc.vector.scalar_tensor_tensor(
                    junkf[:64, :CH], a3kv[:64, k * CH:(k + 1) * CH], 1.0,
                    a3kv[:64, k * CH:(k + 1) * CH], OP.mult, OP.mult,
                    accum_out=kn2[:, k:k + 1])

            # ---------------- gram phase: G_qk ----------------
            def _cp(eng, dst, srcap):
                if eng is nc.scalar:
                    eng.copy(dst, srcap)
                else:
                    eng.tensor_copy(dst, srcap)

            Gq = ppsG.tile([64, 64], F32, tag="gpsum")
            NCH = NQ // 128  # 128 q chunks
            for i in range(NCH):
                tps = ppsT.tile([128, 128], BF, tag="tps")
                nc.tensor.transpose(tps[:], a3qp[:, i * 128:(i + 1) * 128],
                                    s_idn[:])
                tq = ptchk.tile([128, 128], BF, tag="tq")
                _cp([nc.vector, nc.scalar][i % 2], tq[:], tps[:])
                tps0 = ppsT.tile([128, 128], BF, tag="tps")
                nc.tensor.transpose(tps0[:, :64],
                                    a3kv[:64, i * 128:(i + 1) * 128],
                                    s_idn[:64, :64])
                tk0 = ptchk.tile([128, 64], BF, tag="tk0")
                _cp([nc.scalar, nc.vector][i % 2], tk0[:], tps0[:, :64])
                tps1 = ppsT.tile([128, 128], BF, tag="tps")
                nc.tensor.transpose(
                    tps1[:, :64],
                    a3kv[:64, NQ + i * 128:NQ + (i + 1) * 128],
                    s_idn[:64, :64])
                tk1 = ptchk.tile([128, 64], BF, tag="tk1")
                _cp([nc.vector, nc.scalar][(i + 1) % 2], tk1[:], tps1[:, :64])
                nc.tensor.matmul(Gq[:], tq[:, 0:64], tk0[:],
                                 start=(i == 0), stop=False,
                                 skip_group_check=True)
                nc.tensor.matmul(Gq[:], tq[:, 64:128], tk1[:],
                                 start=False, stop=(i == NCH - 1),
                                 skip_group_check=True)

            # ---------------- pack + allreduce 2 ----------------
            nc.gpsimd.memset(av2[:], 0.0)
            nc.vector.tensor_copy(av2[:64, 0:64], Gq[:])
            nc.vector.tensor_reduce(av2[:, 64:65], qn2[:], AX.X, OP.add)
            nc.vector.tensor_reduce(av2[:64, 65:66], kn2[:], AX.X, OP.add)
            d_av = pdram.tile([128, 66], F32)
            d_avr = pdram.tile([128, 66], F32)
            nc.gpsimd.dma_start(d_av[:], av2[:])
            nc.gpsimd.collective_compute(
                "AllReduce", OP.add,
                replica_groups=[[0, 1], [2, 3], [4, 5], [6, 7]],
                ins=[d_av.opt()], outs=[d_avr.opt()])
            avr = pers.tile([128, 66], F32)
            nc.gpsimd.dma_start(avr[:], d_avr[:])

            # ---------------- tiny attention ----------------
            qtmp = ptiny.tile([64, 1], F32, tag="qtmp")
            nc.sync.dma_start(qtmp[:], avr[64:128, 64:65])
            nrm2 = ptiny.tile([64, 2], F32, tag="nrm2")
            nc.vector.tensor_add(nrm2[:, 0:1], avr[:64, 64:65], qtmp[:])
            nc.vector.tensor_copy(nrm2[:, 1:2], avr[:64, 65:66])
            rn = ptiny.tile([64, 2], F32, tag="rn")
            nc.vector.reciprocal(rn[:], nrm2[:])
            yn = ptiny.tile([64, 2], F32, tag="yn")
            nc.scalar.sqrt(yn[:], rn[:])
            tn = ptiny.tile([64, 2], F32, tag="tn")
            nc.vector.tensor_mul(tn[:], yn[:], yn[:])
            nc.vector.tensor_mul(tn[:], tn[:], nrm2[:])
            nc.vector.tensor_scalar(tn[:], tn[:], -0.5, 1.5, OP.mult, OP.add)
            nc.vector.tensor_mul(yn[:], yn[:], tn[:])
            rq = ptiny.tile([64, 1], F32, tag="rq")
            nc.vector.tensor_mul(rq[:], yn[:, 0:1], s_tau[:])
            # rk broadcast across free dim
            rkT = ppsG.tile([1, 64], F32, tag="gpsum")
            nc.tensor.transpose(rkT[:], yn[:, 1:2], s_idnf[:])
            rkrow = ptiny.tile([1, 64], F32, tag="rkrow")
            nc.vector.tensor_copy(rkrow[:], rkT[:])
            rkbc = ptiny.tile([64, 64], F32, tag="rkbc")
            nc.gpsimd.partition_broadcast(rkbc[:], rkrow[:])
            # logits
            L = ptiny.tile([64, 64], F32, tag="L")
            nc.vector.tensor_copy(L[:], avr[:64, 0:64])
            nc.vector.tensor_scalar_mul(L[:], L[:], rq[:])
            nc.vector.tensor_mul(L[:], L[:], rkbc[:])
            nc.scalar.activation(L[:], L[:], AF.Exp)
            nc.vector.tensor_mul(L[:], L[:], s_bmask[:])
            rs = ptiny.tile([64, 1], F32, tag="rs")
            nc.vector.tensor_reduce(rs[:], L[:], AX.X, OP.add)
            nc.vector.reciprocal(rs[:], rs[:])
            nc.vector.tensor_scalar_mul(L[:], L[:], rs[:])
            # W2 = Abd^T @ P^T  -> [vc, o]
            w2ps = ppsG.tile([64, 64], F32, tag="gpsum")
            nc.tensor.matmul(w2ps[:], L[:], s_projT[:], start=True, stop=True)
            w2sb = ptiny.tile([64, 64], BF, tag="w2sb")
            nc.scalar.copy(w2sb[:], w2ps[:])
            W2big = pers.tile([128, 64], BF)
            nc.gpsimd.memset(W2big[:64, :], 0.0)
            nc.sync.dma_start(W2big[64:128, :], w2sb[:])

            # ---------------- out = (P@Abd) @ v ----------------
            for k in range(NKV // 512):
                ps = pps.tile([64, 512], F32)
                nc.tensor.matmul(ps[:], W2big[:],
                                 a3kv[:, k * 512:(k + 1) * 512],
                                 start=True, stop=True)
                osb = posb.tile([64, 512], BF, tag="osb")
                nc.scalar.copy(osb[:], ps[:])
                nc.sync.dma_start(out_d[:, k * 512:(k + 1) * 512], osb[:])

    nc.compile()
    _CACHE["nc"] = nc
    return nc


def _pack_weights(inputs, sx, sy):
    """Build the shared [128, BPT] weight-bytes template + per-core masks."""
    bf16 = ml_dtypes.bfloat16

    def z(*s):
        return np.zeros(s, np.float32)

    kv_w = np.asarray(inputs["kv_w"], np.float32)[:, :, 0, 0]
    q_w = np.asarray(inputs["q_w"], np.float32)[:, :, 0, 0]
    proj_w = np.asarray(inputs["proj_w"], np.float32)[:, :, 0, 0]
    kv1 = np.asarray(inputs["kv_c1_w"], np.float32)[:, :, 0, 0]
    q1 = np.asarray(inputs["q_c1_w"], np.float32)[:, :, 0, 0]

    def blockdiag(a):
        o = z(128, 128)
        o[:64, :64] = a
        o[64:, 64:] = a
        return o

    w5kv_ = np.asarray(inputs["kv_c0_w"], np.float32)[:, 0].reshape(128, 25)
    w3kv_ = np.asarray(inputs["kv_cs_w"], np.float32)[:, 0].reshape(128, 9)
    w5q1 = np.asarray(inputs["q_c0_w"], np.float32)[:, 0].reshape(64, 25)
    w3q1 = np.asarray(inputs["q_cs_w"], np.float32)[:, 0].reshape(64, 9)
    w5q_ = np.concatenate([w5q1, w5q1], 0)
    w3q_ = np.concatenate([w3q1, w3q1], 0)

    def dup(v):
        return np.concatenate([v, v], 0).reshape(128, 1)

    def padc(a, cols):
        o = np.zeros((a.shape[0], cols), a.dtype)
        o[:, :a.shape[1]] = a
        return o

    ind = z(128, 4)
    ind[0:64, 0] = 1.0
    ind[64:128, 1] = 1.0
    pp = np.arange(128) % 64
    ind[pp < 32, 2] = 1.0
    ind[pp >= 32, 3] = 1.0
    bckv = z(4, 128)
    bckv[0, 0:64] = 1.0
    bckv[1, 64:128] = 1.0
    bcq = z(4, 128)
    bcq[2, pp < 32] = 1.0
    bcq[3, pp >= 32] = 1.0
    cntr = np.array([[1.0 / (64 * H * W)], [1.0 / (64 * H * W)],
                     [1.0 / (32 * H * W)], [1.0 / (32 * H * W)]], np.float32)
    bm = z(64, 64)
    for h in range(4):
        bm[h * 16:(h + 1) * 16, h * 16:(h + 1) * 16] = 1.0

    vals = {
        "kvwT": (kv_w.T / sy).astype(bf16),
        "kv1wT": kv1.T.astype(bf16),
        "qwT2": blockdiag(q_w.T / sx).astype(bf16),
        "q1wT2": blockdiag(q1.T).astype(bf16),
        "w5kv": padc(w5kv_, 26), "w3kv": padc(w3kv_, 10),
        "w5q": padc(w5q_, 26), "w3q": padc(w3q_, 10),
        "bkv0": np.asarray(inputs["kv_c0_b"], np.float32).reshape(128, 1),
        "bkvs": np.asarray(inputs["kv_cs_b"], np.float32).reshape(128, 1),
        "bkv1": np.asarray(inputs["kv_c1_b"], np.float32).reshape(128, 1),
        "bq0": dup(np.asarray(inputs["q_c0_b"], np.float32)),
        "bqs": dup(np.asarray(inputs["q_cs_b"], np.float32)),
        "bq1": dup(np.asarray(inputs["q_c1_b"], np.float32)),
        "g_kv": np.asarray(inputs["kv_gn_g"], np.float32).reshape(128, 1),
        "be_kv": np.asarray(inputs["kv_gn_b"], np.float32).reshape(128, 1),
        "g_q": dup(np.asarray(inputs["q_gn_g"], np.float32)),
        "be_q": dup(np.asarray(inputs["q_gn_b"], np.float32)),
        "ind": ind, "bc_kv": bckv, "bc_q": bcq, "cntr": cntr,
        "tau64": np.repeat(np.asarray(inputs["temperature"],
                                      np.float32).reshape(4), 16)
        .reshape(64, 1).copy(),
        "bmask": bm,
        "idn": np.eye(128, dtype=np.float32).astype(bf16),
        "idnf": np.eye(64, dtype=np.float32),
        "projT": proj_w.T.copy(),
    }

    wt = np.zeros((128, BPP - OWT), np.int8)
    for name, p, nb in _WSPEC:
        if name.startswith("m0"):
            continue
        a = np.ascontiguousarray(vals[name])
        bts = a.view(np.int8).reshape(p, -1)
        o = _WOFF[name] - OWT
        wt[:p, o:o + bts.shape[1]] = bts
    return wt


def _prep(inputs):
    x = np.asarray(inputs["x"], np.float32)
    y = np.asarray(inputs["y"], np.float32)
    sx = float(127.0 / max(np.abs(x).max(), 1e-30))
    sy = float(127.0 / max(np.abs(y).max(), 1e-30))
    xq = np.clip(np.rint(x * sx), -127, 127).astype(np.int8)
    yq = np.clip(np.rint(y * sy), -127, 127).astype(np.int8)

    wt = _pack_weights(inputs, sx, sy)

    blobs = []
    for core in range(N_CORES):
        b, half = core // 2, core % 2
        r0 = half * R
        blob = np.zeros((128, BPP), np.int8)
        # ya: rows r0-5 .. r0+R+5 (138-row halo space), split at YSPLIT
        lo, hi = r0 - 5, r0 + R + 5
        slo, shi = max(lo, 0), min(hi, H)
        ya = np.zeros((C, R + 10, W), np.int8)
        ya[:, slo - lo:shi - lo, :] = yq[b, :, slo:shi, :]
        blob[0:64, OYA:OYA + YA_B] = ya[:, :YSPLIT].reshape(64, -1)
        blob[64:128, OYA:OYA + YA_B] = ya[:, YSPLIT:].reshape(64, -1)
        # xa: two packed halves with 5-row halos
        xa = np.zeros((128, R // 2 + 10, W), np.int8)
        for hf in range(2):
            base = r0 + hf * (R // 2)
            lo2, hi2 = base - 5, base + R // 2 + 5
            s2, e2 = max(lo2, 0), min(hi2, H)
            xa[hf * 64:(hf + 1) * 64, s2 - lo2:e2 - lo2, :] = xq[b, :, s2:e2, :]
        blob[:, OXA:OXA + XA_B] = xa.reshape(128, -1)
        # weights template + per-core masks
        blob[:, OWT:] = wt
        f32 = np.float32
        m0t_kv = np.full((128, 1), 0.0 if r0 == 0 else 1.0, f32)
        m0b_kv = np.full((128, 1), 0.0 if r0 + R == H else 1.0, f32)
        mtq = np.ones((128, 1), f32)
        if r0 == 0:
            mtq[0:64] = 0.0
        mbq = np.ones((128, 1), f32)
        if r0 + R == H:
            mbq[64:128] = 0.0
        for name, arr in (("m0t_kv", m0t_kv), ("m0b_kv", m0b_kv),
                          ("m0t_q", mtq), ("m0b_q", mbq)):
            o = _WOFF[name]
            blob[:, o:o + 4] = arr.view(np.int8)
        blobs.append(blob)
    return blobs


def _get_runner(nc):
    if "runner" in _CACHE:
        return _CACHE["runner"]
    import jax
    import jax.numpy as jnp
    from jax.sharding import Mesh, PartitionSpec, NamedSharding
    from jax.experimental.shard_map import shard_map
    from concourse import mybir
    from concourse.bass2jax import (_bass_exec_p, install_neuronx_cc_hook,
                                    partition_id_tensor)
    try:
        jax.config.update("jax_compilation_cache_dir", "/var/tmp/jax_cache")
        jax.config.update("jax_persistent_cache_min_entry_size_bytes", -1)
        jax.config.update("jax_persistent_cache_min_compile_time_secs", 0)
    except Exception:
        pass
    install_neuronx_cc_hook()

    partition_name = (nc.partition_id_tensor.name
                      if nc.partition_id_tensor else None)
    in_names, out_names, out_avals = [], [], []
    for alloc in nc.m.functions[0].allocations:
        if not isinstance(alloc, mybir.MemoryLocationSet):
            continue
        name = alloc.memorylocations[0].name
        if alloc.kind == "ExternalInput":
            if name != partition_name:
                in_names.append(name)
        elif alloc.kind == "ExternalOutput":
            out_names.append(name)
            shape = tuple(alloc.tensor_shape)
            dtype = mybir.dt.np(alloc.dtype)
            out_avals.append(jax.core.ShapedArray(shape, dtype))
    assert in_names == ["blob"] and out_names == ["out"], (in_names, out_names)
    n_params = len(in_names)
    n_outs = len(out_avals)
    all_names = list(in_names) + list(out_names)
    if partition_name is not None:
        all_names.append(partition_name)
    donate = tuple(range(n_params, n_params + n_outs))

    def _body(*args):
        operands = list(args)
        if partition_name is not None:
            operands.append(partition_id_tensor())
        outs = _bass_exec_p.bind(
            *operands, out_avals=tuple(out_avals), in_names=tuple(all_names),
            out_names=tuple(out_names), lowering_input_output_aliases=(),
            sim_require_finite=True, sim_require_nnan=True, nc=nc)
        return tuple(outs)

    devices = jax.devices()[:N_CORES]
    mesh = Mesh(np.asarray(devices), ("core",))
    sharding = NamedSharding(mesh, PartitionSpec("core"))
    in_specs = (PartitionSpec("core"),) * (n_params + n_outs)
    out_specs = (PartitionSpec("core"),) * n_outs
    sharded = jax.jit(
        shard_map(_body, mesh=mesh, in_specs=in_specs, out_specs=out_specs,
                  check_rep=False),
        donate_argnums=donate, keep_unused=True)

    oshape = out_avals[0].shape
    odtype = out_avals[0].dtype
    zf = jax.jit(
        lambda: jnp.zeros((N_CORES * oshape[0],) + oshape[1:], odtype),
        out_shardings=sharding)

    runner = {
        "jax": jax, "devices": devices, "sharding": sharding,
        "sharded": sharded, "zf": zf, "oshape": oshape,
    }
    _CACHE["runner"] = runner
    return runner


def kernel(**inputs):
    nc = _build()
    r = _get_runner(nc)
    jax = r["jax"]

    blobs = _prep(inputs)

    # 8 async device puts (one blob per core), assembled into a global array
    parts = [jax.device_put(blobs[c], r["devices"][c])
             for c in range(N_CORES)]
    gblob = jax.make_array_from_single_device_arrays(
        (N_CORES * 128, BPP), r["sharding"], parts)
    zeros = r["zf"]()

    out = r["sharded"](gblob, zeros)[0]
    o_np = np.asarray(out)   # [8*64, NKV] bf16

    global _LAST_EXEC_NS
    _LAST_EXEC_NS = None
    import kernel as _self
    _self._LAST_EXEC_NS = None

    res = np.empty((B, C, H, W), np.float32)
    for core in range(N_CORES):
        b, half = core // 2, core % 2
        res[b, :, half * R:(half + 1) * R, :] = \
            o_np[core * 64:(core + 1) * 64].reshape(C, R, W).astype(np.float32)
    return res
